# revision 52
# baseline (speedup 1.0000x reference)
"""CapsuleLayer1d (dynamic routing) Trainium2 Bass kernel.

Problem: x[4096,64,16] f32, affine_w[32,64,16,16] f32 ->
  u_hat = einsum('bni,ondi->bond', x, W); 3 routing iterations
  (softmax over o, weighted sum over n, squash, logit update) -> out[4096,32,16] f32.

Device program (pure data parallel over 8 cores, 512 samples each):
 - Partition layout: batch on the 128 SBUF partitions; per-sample tensors in the
   free dimension.  4 tiles of 128 samples per core.
 - u_hat computed on the PE as 64 per-n matmuls (K=DIN=16), distributed over the
   four 32-row PE strips via tile_position; accumulated fp32 in PSUM, evacuated
   to SBUF as fp16 in (o, d, n) order (n innermost).
 - Iteration-0 weighted sum (uniform c=1/32) is one extra K=128 PSUM-accumulated
   matmul chain against W/32 pre-arranged on (n,i) partitions.
 - Routing contractions (sum over n with softmax weights, sum over d against v)
   are DVE tensor_tensor + tensor_reduce passes over free-dim views; softmax and
   squash are per-partition free-dim ops (exp/ln on ACT, reciprocal on DVE).
 - All input reshaping/transposition/casting is done host-side in numpy (free).

Dispatch layer: the axon tunnel's multi-device shard_map execute path costs
~100us per (argument x device) shard per call plus a ~5ms fixed cost per
8-device sharded dispatch, while independent single-device executes overlap
almost perfectly across devices.  So each core gets its OWN single-device
fast-dispatch executable (bass_effect suppressed -> C++ no-token dispatch
path); a kernel() call issues CPG async single-device calls and blocks once.
Weights are baked into the NEFF as Const tensors (loaded to HBM once at model
load) so the only per-call IO is the dense fp16 x layout in and the fp16
output back (~12 MB/call total, the tunnel's per-call floor).  Consecutive
calls rotate between GROUPS disjoint core groups, each computing the full
batch, so back-to-back calls pipeline: one group executes while the other's
per-execute fixed costs are absorbed.
"""

import hashlib
from contextlib import ExitStack

import numpy as np

B, O, N, DOUT, DIN = 4096, 32, 64, 16, 16
NCORES = 8
GROUPS = 2                 # device groups; consecutive calls rotate groups
CPG = NCORES // GROUPS     # cores used per call
BC = B // CPG              # samples per core per call
P = 128                    # partitions (samples per tile)
NT = BC // P               # tiles of 128 samples per core
OD = O * DOUT     # 512
ON = O * N        # 2048
EPS = 1e-8
# o-dim routing chunks: (offset, size, on_gpsimd) — GPSIMD is ~1.56x slower
# per element than the DVE 2x mode, so it gets proportionally smaller chunks
# (18 o's on DVE, 14 on GPSIMD ~ the measured 21:27 per-o cost ratio).
CHUNKS = [(0, 9, False), (9, 7, True), (16, 9, False), (25, 7, True)]


def emit(tc, io, NT):
    import concourse.bass as bass  # noqa: F401
    from concourse import mybir

    dt = mybir.dt
    Alu = mybir.AluOpType
    Act = mybir.ActivationFunctionType
    X = mybir.AxisListType.X
    nc = tc.nc
    bf, f32 = dt.float16, dt.float32
    EXPB = -8.0  # constant softmax-logit bias so exp() fits fp16 comfortably

    with ExitStack() as ctx:
        consts = ctx.enter_context(tc.tile_pool(name="consts", bufs=1))
        x_pool = ctx.enter_context(tc.tile_pool(name="xt", bufs=1))
        u_pool = ctx.enter_context(tc.tile_pool(name="u", bufs=2))
        ch_pool = ctx.enter_context(tc.tile_pool(name="chunk", bufs=1))
        rt_pool = ctx.enter_context(tc.tile_pool(name="rt", bufs=1))
        sm_pool = ctx.enter_context(tc.tile_pool(name="small", bufs=1))
        out_pool = ctx.enter_context(tc.tile_pool(name="outp", bufs=2))
        sv_pool = ctx.enter_context(tc.tile_pool(name="sv", bufs=1))
        # double-buffered so tile t+1's iteration-0 squash/v/dots can run in
        # tile t's softmax-barrier gaps instead of waiting for t's last read
        pipe2 = ctx.enter_context(tc.tile_pool(name="pipe2", bufs=2))
        psum_u = ctx.enter_context(tc.tile_pool(name="psum_u", bufs=2, space="PSUM"))
        psum_s0 = ctx.enter_context(tc.tile_pool(name="psum_s0", bufs=1, space="PSUM"))

        # w_rhs/xt_a ship dense ([64, ...], row 16s+j = strip s, i=j) and are
        # expanded to the PE strip layout (partitions 32s+j, upper 16 rows of
        # each strip unused -- PE lhsT/rhs must start at the strip base) with
        # one DMA per strip.  xt is streamed per tile (bufs=1; the DMA for
        # tile t+1 starts as soon as tile t's last matmul has read the slot).
        expb = consts.tile([P, 1], f32)
        nc.gpsimd.memset(expb, EXPB)
        w_sb = consts.tile([P, 16 * OD], bf)
        wsrc = io["w_rhs"].rearrange("(s j) f -> s j f", s=4)
        xa = io["xt_a"].rearrange("(s j) t f -> s j t f", s=4)
        for s in range(4):
            nc.sync.dma_start(out=w_sb[32 * s:32 * s + 16], in_=wsrc[s])

        for t in range(NT):
            xt_sb = x_pool.tile([P, 16 * P], bf, tag="xt")
            for s in range(4):
                nc.sync.dma_start(
                    out=xt_sb[32 * s:32 * s + 16], in_=xa[s][:, t])

            u = u_pool.tile([P, O * DOUT * N], bf, tag="u")  # (o, d, n), n innermost
            u4 = u.rearrange("p (o d n) -> p o d n", o=O, d=DOUT)

            # u_hat per-n matmuls on the four PE row strips, evacuated on ACT
            # in groups of 2 n's (one 2-bank PSUM tile per group); a second
            # PE pass below accumulates the iteration-0 weighted sum
            # s0 = sum_n u_hat into four per-strip PSUM banks, so no extra
            # DVE work or second x layout input is needed.
            s0p = psum_s0.tile([P, 4, OD], f32, tag="s0")
            for q in range(N // 2):
                pu = psum_u.tile([P, 2, OD], f32, tag="pu", name="pu")
                for jj in range(2):
                    n = 2 * q + jj
                    st, j = n // 16, n % 16
                    lhsT = xt_sb[32 * st:32 * st + 16, j * P:(j + 1) * P]
                    rhs = w_sb[32 * st:32 * st + 16, j * OD:(j + 1) * OD]
                    nc.tensor.matmul(
                        pu[:, jj], lhsT=lhsT, rhs=rhs,
                        start=True, stop=True,
                        tile_position=(32 * st, 0),
                    )
                dstv = u4[:, :, :, 2 * q:2 * q + 2]        # [P, O, D, 2]
                srcv = pu.rearrange("p n (o d) -> p o d n", o=O)
                # ACT evacuates; on the first tile (nothing else running yet)
                # the DVE takes half so the exposed startup fill is shorter.
                if t == 0 and q % 2 == 1:
                    nc.vector.tensor_copy(out=dstv, in_=srcv)
                else:
                    nc.scalar.copy(out=dstv, in_=srcv)
            # second PE pass: s0 = sum_n u_hat as four contiguous 16-matmul
            # PSUM accumulation chains, one per strip.  walrus rejects
            # accumulation chains that are interleaved with other matmuls or
            # span two tile_positions, so this stays a separate pass.
            for n in range(N):
                st, j = n // 16, n % 16
                nc.tensor.matmul(
                    s0p[:, st],
                    lhsT=xt_sb[32 * st:32 * st + 16, j * P:(j + 1) * P],
                    rhs=w_sb[32 * st:32 * st + 16, j * OD:(j + 1) * OD],
                    start=(j == 0), stop=(j == 15),
                    tile_position=(32 * st, 0),
                )

            # ---- routing state tiles ----
            logits = rt_pool.tile([P, ON], f32, tag="logits")  # (o, n)
            lo3 = logits.rearrange("p (o n) -> p o n", o=O)
            ex = rt_pool.tile([P, ON], bf, tag="ex")
            ex3 = ex.rearrange("p (o n) -> p o n", o=O)
            s_sb = pipe2.tile([P, OD], f32, tag="s")
            s3 = s_sb.rearrange("p (o d) -> p o d", o=O)
            sq = pipe2.tile([P, OD], bf, tag="sq")
            sq3 = sq.rearrange("p (o d) -> p o d", o=O)
            vbf = pipe2.tile([P, OD], bf, tag="v")
            v3 = vbf.rearrange("p (o d) -> p o d", o=O)
            Zt = sm_pool.tile([P, N], bf, tag="Z")
            Zi = sm_pool.tile([P, N], bf, tag="Zi")
            Zp = sm_pool.tile([P, 4, N], bf, tag="Zp")
            r2 = pipe2.tile([P, O], f32, tag="r2")
            lnr = pipe2.tile([P, O], f32, tag="lnr")
            rr = pipe2.tile([P, O], f32, tag="rr")
            reps = pipe2.tile([P, O], f32, tag="reps")
            denom = pipe2.tile([P, O], f32, tag="denom")
            dinv = pipe2.tile([P, O], f32, tag="dinv")
            alpha = pipe2.tile([P, O], f32, tag="alpha")
            alpha_b = alpha.unsqueeze(2).broadcast_to([P, O, DOUT])

            def squash_scalars(sl):
                # r2 [P,osz] -> alpha [P,osz];  alpha = r2/((1+r2)(r+eps)),
                # r = sqrt(r2) via exp(0.5*ln(r2)) (one ACT table set).
                # Sliced per o-group so each group's dot/v work can start
                # without waiting for the other groups' s.
                nc.scalar.activation(out=lnr[:, sl], in_=r2[:, sl], func=Act.Ln)
                nc.scalar.activation(out=rr[:, sl], in_=lnr[:, sl],
                                     func=Act.Exp, scale=0.5)
                nc.vector.tensor_scalar_add(out=reps[:, sl], in0=rr[:, sl],
                                            scalar1=EPS)
                nc.vector.scalar_tensor_tensor(
                    out=denom[:, sl], in0=r2[:, sl], scalar=1.0, in1=reps[:, sl],
                    op0=Alu.add, op1=Alu.mult,
                )
                nc.vector.reciprocal(out=dinv[:, sl], in_=denom[:, sl])
                nc.vector.tensor_tensor(out=alpha[:, sl], in0=r2[:, sl],
                                        in1=dinv[:, sl], op=Alu.mult)

            def tree_n(prod, dst, eng):
                # prod [P, G, D, N] fp16 -> dst [P, G, D] f32, sum over innermost n
                # fp16 tree adds run the DVE 2x mode; tensor_reduce would be 1x.
                # Whole chunk stays on one engine (DVE or the idle GPSIMD).
                sz = N // 2
                while sz >= 2:
                    eng.tensor_tensor(
                        out=prod[:, :, :, :sz], in0=prod[:, :, :, :sz],
                        in1=prod[:, :, :, sz:2 * sz], op=Alu.add)
                    sz //= 2
                eng.tensor_tensor(
                    out=dst, in0=prod[:, :, :, 0], in1=prod[:, :, :, 1], op=Alu.add)

            def dot_group(g, o0, osz, eng, add):
                # lo3[p,o,n] (+)= sum_d u[p,o,d,n] * v[p,o,d] for the group's
                # o-slice, then the NEXT softmax's exp + partial-Z, so the
                # following iteration's joins pipeline with the dot chunks.
                # v pre-duplicated into pairs (v2x) so the broadcast has a
                # step-1 innermost dim -> the mult runs the DVE 2x mode.
                # The final tree level accumulates straight into the logits
                # (two adds) so no separate dot buffer exists.
                sl = slice(o0, o0 + osz)
                v2x = sv_pool.tile([P, osz, DOUT, 2], bf,
                                   tag=f"v2x{eng is nc.gpsimd}", name="v2x")
                nc.vector.tensor_copy(
                    out=v2x,
                    in_=v3[:, sl].unsqueeze(3).broadcast_to([P, osz, DOUT, 2]))
                ug = u4[:, sl].rearrange("p o d (h two) -> p o d h two", two=2)
                vg = (v2x
                      .unsqueeze(3)
                      .broadcast_to([P, osz, DOUT, N // 2, 2]))
                prod = ch_pool.tile([P, osz, DOUT, N], bf,
                                    tag=f"prod{eng is nc.gpsimd}", name="prod")
                prod5 = prod.rearrange("p o d (h two) -> p o d h two", two=2)
                eng.tensor_tensor(out=prod5, in0=ug, in1=vg, op=Alu.mult)
                sz = DOUT // 2
                while sz >= 2:
                    eng.tensor_tensor(
                        out=prod[:, :, :sz], in0=prod[:, :, :sz],
                        in1=prod[:, :, sz:2 * sz], op=Alu.add)
                    sz //= 2
                if add:
                    eng.tensor_tensor(out=lo3[:, sl], in0=lo3[:, sl],
                                      in1=prod[:, :, 0], op=Alu.add)
                    eng.tensor_tensor(out=lo3[:, sl], in0=lo3[:, sl],
                                      in1=prod[:, :, 1], op=Alu.add)
                else:
                    eng.tensor_tensor(out=lo3[:, sl], in0=prod[:, :, 0],
                                      in1=prod[:, :, 1], op=Alu.add)
                nc.scalar.activation(
                    out=ex3[:, sl], in_=lo3[:, sl], func=Act.Exp, bias=expb)
                with nc.allow_low_precision(
                        reason="fp16 softmax partials; <=32 biased-exp terms"):
                    nc.vector.tensor_reduce(
                        out=Zp[:, g], in_=ex3[:, sl].transpose([0, 2, 1]),
                        axis=X, op=Alu.add)

            def s_group(o0, osz, eng, with_v):
                # s = sum_n c*u for the o-slice, squash partials + scalars,
                # and (except on the last iteration) the v for the dots --
                # all group-local so the four groups pipeline freely.
                sl = slice(o0, o0 + osz)
                Zb = Zi.unsqueeze(1).broadcast_to([P, osz, N])
                c_t = sv_pool.tile([P, osz, N], bf,
                                   tag=f"c{eng is nc.gpsimd}", name="c")
                eng.tensor_tensor(
                    out=c_t, in0=ex3[:, sl], in1=Zb, op=Alu.mult)
                cg = c_t.unsqueeze(2).broadcast_to([P, osz, DOUT, N])
                cu = ch_pool.tile([P, osz, DOUT, N], bf,
                                  tag=f"prod{eng is nc.gpsimd}", name="prod")
                eng.tensor_tensor(out=cu, in0=u4[:, sl], in1=cg, op=Alu.mult)
                tree_n(cu, s3[:, sl], eng)
                nc.vector.tensor_tensor(
                    out=sq3[:, sl], in0=s3[:, sl], in1=s3[:, sl], op=Alu.mult)
                nc.vector.tensor_reduce(
                    out=r2[:, sl], in_=sq3[:, sl], axis=X, op=Alu.add)
                squash_scalars(sl)
                if with_v:
                    nc.vector.tensor_tensor(
                        out=v3[:, sl], in0=s3[:, sl],
                        in1=alpha_b[:, sl], op=Alu.mult)

            # ==== iteration 0 ====
            # s_sb holds 32*s0 (raw sum over n); fold the 1/32 into the squash
            # scalars instead: r2 *= 1/1024, alpha *= 1/32.
            # (the Pool engine has no PSUM port, so the joins stay on DVE;
            # only the square -- SBUF-only -- runs on Pool)
            nc.scalar.copy(out=s_sb, in_=s0p[:, 0])
            for hb in (1, 2, 3):
                nc.vector.tensor_tensor(
                    out=s_sb, in0=s_sb, in1=s0p[:, hb], op=Alu.add)
            nc.gpsimd.tensor_tensor(out=sq, in0=s_sb, in1=s_sb, op=Alu.mult)
            nc.vector.tensor_reduce(out=r2, in_=sq3, axis=X, op=Alu.add)
            nc.vector.tensor_scalar_mul(out=r2, in0=r2, scalar1=1.0 / 1024.0)
            squash_scalars(slice(0, O))
            nc.vector.tensor_scalar_mul(out=alpha, in0=alpha, scalar1=1.0 / 32.0)
            nc.vector.tensor_tensor(out=v3, in0=s3, in1=alpha_b, op=Alu.mult)
            # b1 = <u, v0> (b0 == 0); fused per o-group with the next softmax's
            # exp and per-group partial Z so the joins pipeline.
            for g, (o0, osz, gps) in enumerate(CHUNKS):
                dot_group(g, o0, osz, nc.gpsimd if gps else nc.vector, add=False)

            for it in (1, 2):
                # softmax over o (no max subtraction; logits are O(10)):
                # join the four partial sums, invert, then per-group c.
                nc.vector.tensor_tensor(
                    out=Zt, in0=Zp[:, 0], in1=Zp[:, 1], op=Alu.add)
                nc.vector.tensor_tensor(
                    out=Zt, in0=Zt, in1=Zp[:, 2], op=Alu.add)
                nc.vector.tensor_tensor(
                    out=Zt, in0=Zt, in1=Zp[:, 3], op=Alu.add)
                with nc.allow_low_precision(
                        reason="fp16 softmax normalizer; tolerance 2e-2"):
                    nc.vector.reciprocal(out=Zi, in_=Zt)
                for g, (o0, osz, gps) in enumerate(CHUNKS):
                    s_group(o0, osz, nc.gpsimd if gps else nc.vector,
                            with_v=(it == 1))
                if it == 1:
                    # dot + logit update + next softmax exp/partial-Z, per group
                    for g, (o0, osz, gps) in enumerate(CHUNKS):
                        dot_group(g, o0, osz, nc.gpsimd if gps else nc.vector,
                                  add=True)
                else:
                    out_sb = out_pool.tile([P, OD], bf, tag="out")
                    o3 = out_sb.rearrange("p (o d) -> p o d", o=O)
                    nc.vector.tensor_tensor(out=o3, in0=s3, in1=alpha_b, op=Alu.mult)
                    nc.sync.dma_start(out=io["out"][t * P:(t + 1) * P, :], in_=out_sb)


def _legalize_mm_waits(nc):
    """Several ISA structs have a single sync-wait slot; Tile can emit
    instructions with 2+ waits (pool-slot recycle + cross-engine RAW). Split
    the excess waits onto a chain of inserted same-engine single-wait nops
    (equivalent under in-order engine execution)."""
    from concourse import mybir

    f = nc.m.functions[0]
    for blk in f.blocks:
        out = []
        changed = False
        for ins in blk.instructions:
            si = ins.sync_info
            if si is not None and si.on_wait and len(si.on_wait) > 1 \
                    and ins.engine != mybir.EngineType.Unassigned:
                waits = list(si.on_wait)
                for w in waits[:-1]:
                    nop = mybir.InstNoOp(
                        name=nc.get_next_instruction_name(),
                        sync_info=mybir.SyncInfo(on_wait=[w], on_update=[]),
                        bass_nofuse=True,
                        engine=ins.engine,
                    )
                    out.append(nop)
                ins.sync_info = mybir.SyncInfo(
                    on_wait=[waits[-1]], on_update=list(si.on_update or []))
                changed = True
            out.append(ins)
        if changed:
            blk.instructions = out
    return nc


def build(w_rhs, legalize=True, partition_id=False):
    import concourse.bass as bass
    import concourse.tile as tile
    from concourse import mybir

    dt = mybir.dt
    nc = bass.Bass("TRN2", debug=False, enable_partition_id=partition_id)
    io = {
        "xt_a": nc.dram_tensor("xt_a", [64, NT, 16 * P], dt.float16,
                               kind="ExternalInput").ap(),
        "w_rhs": nc.inline_tensor(np.ascontiguousarray(w_rhs), name="w_rhs_c").ap(),
        "out": nc.dram_tensor("out", [NT * P, OD], dt.float16,
                              kind="ExternalOutput").ap(),
    }
    with tile.TileContext(nc) as tc:
        emit(tc, io, NT)
    if legalize:
        _legalize_mm_waits(nc)  # HW-only: CoreSim lacks bookkeeping for the
        # injected nops, and the transform is semantics-preserving.
    return nc


def prep_weights(affine_w):
    f16 = np.float16
    W = np.asarray(affine_w, np.float32)  # [O,N,D,I]

    # w_rhs dense [64, 16, OD]: row 16s+j holds W[o, 16s+nn, d, i=j] at free
    # (nn, o*16+d); DMA'd per strip into partitions 32s+j on device.
    w_rhs = np.zeros((64, 16, OD), np.float32)
    # W arranged [I, N, O, D]:
    Wt = W.transpose(3, 1, 0, 2)  # [I, N, O, D]
    for s in range(4):
        # rows 16s..16s+15  <- i=j, n block 16s..16s+16
        w_rhs[16 * s:16 * s + 16] = Wt[:, 16 * s:16 * s + 16].reshape(16, 16, OD)
    return w_rhs.reshape(64, 16 * OD).astype(f16)


def prep_x(x_c, NT):
    """Per-core x [BC,N,I] -> xt_a [64, NT, 16*128] (dense)."""
    f16 = np.float16
    xt = np.asarray(x_c, np.float32).transpose(1, 2, 0)  # [N, I, BC]

    xt_a = np.zeros((64, NT, 16, P), np.float32)
    for s in range(4):
        # row 16s+j = i=j of strip s; free (nn, b)
        blk = xt[16 * s:16 * s + 16]               # [16n, 16i, BC]
        blk = blk.transpose(1, 0, 2)               # [16i, 16n, BC]
        xt_a[16 * s:16 * s + 16] = blk.reshape(16, 16, NT, P).transpose(0, 2, 1, 3)
    return xt_a.reshape(64, NT, 16 * P).astype(f16)


_CACHE = {}


def _ensure_built(W):
    """Build the Bass program (weights baked in as NEFF consts) and one
    single-device fast-dispatch executable per NeuronCore."""
    whash = hashlib.sha1(np.ascontiguousarray(W, np.float32).tobytes()).hexdigest()
    if _CACHE.get("whash") == whash:
        return
    import jax
    from concourse import bass2jax
    from concourse import mybir

    w_rhs = prep_weights(W)
    nc = build(w_rhs)
    bass2jax.install_neuronx_cc_hook()

    in_specs, out_names, out_avals = [], [], []
    for alloc in nc.m.functions[0].allocations:
        if not isinstance(alloc, mybir.MemoryLocationSet):
            continue
        name = alloc.memorylocations[0].name
        if alloc.kind == "ExternalInput":
            in_specs.append((name, tuple(alloc.tensor_shape),
                             mybir.dt.np(alloc.dtype)))
        elif alloc.kind == "ExternalOutput":
            out_names.append(name)
            out_avals.append(jax.core.ShapedArray(
                tuple(alloc.tensor_shape), mybir.dt.np(alloc.dtype)))
    in_names = tuple(nm for nm, _, _ in in_specs)

    def _body(*args):
        outs = bass2jax._bass_exec_p.bind(
            *args,
            out_avals=tuple(out_avals),
            in_names=in_names,
            out_names=tuple(out_names),
            lowering_input_output_aliases=(),
            sim_require_finite=True,
            sim_require_nnan=True,
            nc=nc,
        )
        return tuple(outs)

    devs = list(jax.devices()[:NCORES])
    execs = []
    for dev in devs:
        sharding = jax.sharding.SingleDeviceSharding(dev)
        specs = [jax.ShapeDtypeStruct(shp, dtp, sharding=sharding)
                 for _, shp, dtp in in_specs]

        def _compile(specs=specs):
            return jax.jit(_body, keep_unused=True).lower(*specs).compile()

        execs.append(bass2jax.fast_dispatch_compile(_compile))

    _CACHE.update(whash=whash, nc=nc, execs=execs, devs=devs,
                  in_names=in_names)


def kernel(x, affine_w):
    import jax

    x = np.asarray(x, np.float32)
    _ensure_built(np.asarray(affine_w, np.float32))
    execs, devs = _CACHE["execs"], _CACHE["devs"]

    g = _CACHE.get("rr", 0)
    _CACHE["rr"] = (g + 1) % GROUPS
    outs = []
    for i in range(CPG):
        d = g * CPG + i
        xt_a = prep_x(x[i * BC:(i + 1) * BC], NT)
        outs.append(execs[d](jax.device_put(xt_a, devs[d])))
    res = [np.asarray(o[0]) for o in outs]
    return np.concatenate(res, axis=0).reshape(B, O, DOUT).astype(np.float32)


def profile_exec_ns(x, affine_w, iters=512):
    """Per-call device+dispatch time: device-resident inputs, `iters`
    back-to-back full dispatches (8 single-device executes each), one
    block at the end.  iters is large enough to amortize the axon
    tunnel's ~80ms first-result pipeline latency so the number reflects
    the steady-state per-call cost."""
    import time
    import jax

    x = np.asarray(x, np.float32)
    _ensure_built(np.asarray(affine_w, np.float32))
    execs, devs = _CACHE["execs"], _CACHE["devs"]

    shards = [prep_x(x[i * BC:(i + 1) * BC], NT) for i in range(CPG)]
    dargs = [[(jax.device_put(shards[i], devs[g * CPG + i]),)
              for i in range(CPG)] for g in range(GROUPS)]
    jax.block_until_ready(dargs)

    k = [0]

    def call():
        g = k[0] % GROUPS
        k[0] += 1
        return [execs[g * CPG + i](*dargs[g][i]) for i in range(CPG)]

    # warmup (includes NEFF load)
    for _ in range(4):
        outs = call()
    jax.block_until_ready(outs)
    t0 = time.perf_counter()
    outs = None
    for _ in range(iters):
        outs = call()
    jax.block_until_ready(outs)
    dt = time.perf_counter() - t0
    return int(dt / iters * 1e9)


if __name__ == "__main__":
    rng = np.random.default_rng(0)
    x = rng.standard_normal((B, N, DIN), dtype=np.float32)
    W = rng.standard_normal((O, N, DOUT, DIN), dtype=np.float32) * 0.1
    out = kernel(x, W)
    print(out.shape, out.dtype)
